# revision 10
# baseline (speedup 1.0000x reference)
"""ExpertsChooseMlp Trainium2 kernel — all-fp8 DoubleRow pipeline.

Full inputs in, full output out. Sharding: 8 cores = 4 batches x 2 expert-pairs.
Core m handles batch b=m//2 and experts {2g, 2g+1}, g=m%2. Each core computes
pout[T,O] = sum_{e in pair} combine[b,:,e,:] @ mlp_e(dispatch[b,:,e,:]^T @ x[b]);
the host sums the two partials per batch, adds b2 and a rank-1 correction.

All four matmul phases run as fp8e4m3 DoubleRow (K=256/pass, 157 TF/s):
  A: xdT[D,C] = x8^T dm8        (K=T,  64 DR passes/expert)
  B: hT[HE,C] = gelu(w18^T xdT + biasB)   (K=D,  16 passes)
  C: y[C,O]   = hT^T w28        (K=HE, 16 passes)
  D: pout[T,O]= cmT^T y         (K=C,  64 passes over expert pair)
320 matmuls/core at 216ns = ~69us PE-busy.

fp8 accuracy (meas. ~5.7e-3 max rel vs fp32 reference, gate 2e-2) relies on
three host-side corrections that cost zero device time:
 1. x is quantized with sigma-delta error feedback along t: the coherent
    channel (all-positive dispatch weights ~0.5 make the output dominated by
    t-sums of x) would otherwise amplify the sqrt(T) random walk of plain
    rounding error into ~2.4e-2.
 2. w1's quantization error rides the same coherent channel (it multiplies the
    c-mean of xd, constant across capacity slots). biasB = b1 + m @ (w1 - q8(w1))
    with m = mean_c(xd) = (rowsum(dm8)/C) @ x8, computed exactly on host.
 3. w2: same mechanism via the c-mean of h; gelu blocks commuting the mean, so
    m_h is estimated from a 256-row subsample of a host recompute, and the
    correction enters as a host-side rank-1 update R_cm (x) m_h@(w2-q8(w2)).
"""
import sys

sys.path.insert(0, "/opt/trn_rl_repo")

import numpy as np
import ml_dtypes

import concourse.bacc as bacc
import concourse.mybir as mybir
import concourse.tile as tile
from concourse import bass_utils

B, T, D, E, C, HE, O = 4, 2048, 512, 4, 1024, 512, 512
P = 128
nTP = T // 256    # 8  DR passes over T
nMD = D // P      # 4  D-chunks
nMH = HE // P     # 4  HE-chunks
nCC = C // P      # 8  C-chunks
nMT = T // P      # 16 T-chunks (phase D output)
NF = 512          # matmul free dim (one PSUM bank)

F32 = mybir.dt.float32
BF16 = mybir.dt.bfloat16
F8 = mybir.dt.float8e4
GELU = mybir.ActivationFunctionType.Gelu
DR = mybir.MatmulPerfMode.DoubleRow
nCP = nCC // 2    # 4  C pair-chunks for phase D (K=256 per matmul)

F8NP = ml_dtypes.float8_e4m3

_NC = None


def _build():
    nc = bacc.Bacc("TRN2", target_bir_lowering=False, debug=False,
                   enable_asserts=False, num_devices=1)
    # x8 in DR layout: t = tp*256 + i*128 + p  ->  xb[tp, p, i, d]
    xb = nc.dram_tensor("xb", [nTP, P, 2, D], F8, kind="ExternalInput").ap()
    # dm in DR layout per expert: dm[e, tp, p, i, c]
    dm = nc.dram_tensor("dm", [2, nTP, P, 2, C], F8, kind="ExternalInput").ap()
    # cmt planes for phase D: c = kp*256 + i*128 + p -> cmt[e, kp, p, i, t]
    cmt = nc.dram_tensor("cmt", [2, nCP, P, 2, T], F8, kind="ExternalInput").ap()
    # w1 in DR layout: d = pass*256 + i*128 + p -> w1[p, e, pass, i, he]
    w1 = nc.dram_tensor("w1", [P, 2, 2, 2, HE], F8, kind="ExternalInput").ap()
    # w2 in DR layout: he = pass*256 + i*128 + p -> w2[p, e, pass, i, o]
    w2 = nc.dram_tensor("w2", [P, 2, 2, 2, O], F8, kind="ExternalInput").ap()
    # biasB[p, e, mh] = b1 + m@(w1-q8(w1)), he = mh*128 + p
    bb = nc.dram_tensor("bb", [P, 2, nMH], F32, kind="ExternalInput").ap()
    pout = nc.dram_tensor("pout", [T, O], BF16, kind="ExternalOutput").ap()

    with tile.TileContext(nc) as tc:
        with (
            tc.tile_pool(name="const", bufs=1) as const,
            tc.tile_pool(name="dmp", bufs=16) as dmp,
            tc.tile_pool(name="cmp", bufs=8) as cmp_,
            tc.tile_pool(name="inter", bufs=1) as inter,
            tc.tile_pool(name="yp", bufs=2) as yp,
            tc.tile_pool(name="outp", bufs=4) as outp,
            tc.tile_pool(name="psum", bufs=8, space="PSUM") as psp,
        ):
            # ---- warmup memset on Vector so the GpSimd/Sync/Scalar DMA rings
            # aren't blocked behind it ----
            warm = const.tile([P, NF], BF16)
            nc.vector.memset(warm[:], 0.0)

            # ---- DMA ring assignment (each HWDGE queue sustains ~170GB/s;
            # balance so nothing queues behind 12MB of earlier traffic):
            #   sync   : dm[e0] -> cmt[e0] -> pout writes
            #   scalar : x8 -> weights/bias -> dm[e1]
            #   gpsimd : cmt[e1]
            x_sb = const.tile([P, nTP, 2, D], F8)
            for tp in range(nTP):
                nc.scalar.dma_start(x_sb[:, tp, :, :], xb[tp])
            dm_t0 = []
            for tp in range(nTP):
                t_ = dmp.tile([P, 2, C], F8, tag="dm")
                nc.sync.dma_start(t_[:], dm[0, tp])
                dm_t0.append(t_)
            w1_sb = const.tile([P, 2, 2, 2, HE], F8)
            nc.scalar.dma_start(w1_sb[:], w1)
            w2_sb = const.tile([P, 2, 2, 2, O], F8)
            nc.scalar.dma_start(w2_sb[:], w2)
            bb_sb = const.tile([P, 2 * nMH], F32)
            nc.scalar.dma_start(bb_sb[:], bb.rearrange("p e mh -> p (e mh)"))
            dm_t1 = []
            for tp in range(nTP):
                t_ = dmp.tile([P, 2, C], F8, tag="dm")
                nc.scalar.dma_start(t_[:], dm[1, tp])
                dm_t1.append(t_)
            cmt_t = {}
            for ei in range(2):
                eng = nc.sync if ei == 0 else nc.gpsimd
                for kp in range(nCP):
                    t_ = cmp_.tile([P, 2, T], F8, tag="cmt")
                    eng.dma_start(t_[:], cmt[ei, kp])
                    cmt_t[(ei, kp)] = t_

            # ---- HAM warmup: dummy matmuls on zeroed SBUF during the initial
            # DMA wait so real matmuls start at 2.4GHz. ----
            ps_w = psp.tile([P, NF], F32, tag="ps", name="ps_warm")
            for i in range(8):
                nc.tensor.matmul(ps_w[:], warm[:, 0:P], warm[:],
                                 start=(i == 0), stop=(i == 7))

            y_tiles = []
            for ei in range(2):
                dm_t = dm_t0 if ei == 0 else dm_t1

                # ---- phase A: xdT[D, C] = x8^T dm8 (fp8 DR) ----
                # tp-outer: all 8 PSUM banks accumulate in parallel, each dm
                # tile consumed once and released for the next expert prefetch.
                xdt = inter.tile([P, nMD, C], F8, tag="xdt")
                pss = [psp.tile([P, NF], F32, tag="ps", name=f"psa{i}")
                       for i in range(2 * nMD)]
                for tp in range(nTP):
                    for mc in range(nMD):
                        lhsT = x_sb[:, tp, :, mc * P:(mc + 1) * P]
                        nc.tensor.matmul(pss[2 * mc][:], lhsT,
                                         dm_t[tp][:, :, 0:NF],
                                         start=(tp == 0), stop=(tp == nTP - 1),
                                         perf_mode=DR)
                        nc.tensor.matmul(pss[2 * mc + 1][:], lhsT,
                                         dm_t[tp][:, :, NF:C],
                                         start=(tp == 0), stop=(tp == nTP - 1),
                                         perf_mode=DR)
                for ncc in range(2):
                    for mc in range(nMD):
                        nc.vector.tensor_copy(xdt[:, mc, ncc * NF:(ncc + 1) * NF],
                                              pss[2 * mc + ncc][:])

                # ---- phase B: hT[HE, C] = gelu(w18^T xdT + biasB) (fp8 DR) ----
                # ncc-outer so phase C's first C-half unblocks early.
                ht = inter.tile([P, nMH, C], F8, tag="ht")
                for ncc in range(2):
                    sl = slice(ncc * NF, (ncc + 1) * NF)
                    for mh in range(nMH):
                        ps0 = psp.tile([P, NF], F32, tag="ps")
                        for kp in range(2):
                            nc.tensor.matmul(
                                ps0[:],
                                w1_sb[:, ei, kp, :, mh * P:(mh + 1) * P],
                                xdt[:, 2 * kp:2 * kp + 2, sl],
                                start=(kp == 0), stop=(kp == 1), perf_mode=DR)
                        bia = bb_sb[:, ei * nMH + mh:ei * nMH + mh + 1]
                        nc.scalar.activation(ht[:, mh, sl], ps0[:], GELU, bias=bia)

                # ---- phase C: y[C, O] = hT^T w28 (fp8 DR), stored in DR plane
                # layout for phase D: row c = kp*256 + i*128 + p -> y_sb[p,kp,i,:]
                y_sb = yp.tile([P, nCP, 2, O], F8, tag="y")
                for cc in range(nCC):
                    ps = psp.tile([P, NF], F32, tag="ps")
                    for kp in range(2):
                        nc.tensor.matmul(ps[:],
                                         ht[:, 2 * kp:2 * kp + 2, cc * P:(cc + 1) * P],
                                         w2_sb[:, ei, kp, :, :],
                                         start=(kp == 0), stop=(kp == 1),
                                         perf_mode=DR)
                    nc.vector.tensor_copy(y_sb[:, cc // 2, cc % 2, :], ps[:])
                y_tiles.append(y_sb)

            # ---- phase D: pout[T, O] = sum_e cmT_e^T y_e (fp8 DR) ----
            for mt in range(nMT):
                ps = psp.tile([P, NF], F32, tag="ps")
                idx = 0
                for ei in range(2):
                    for kp in range(nCP):
                        nc.tensor.matmul(ps[:],
                                         cmt_t[(ei, kp)][:, :, mt * P:(mt + 1) * P],
                                         y_tiles[ei][:, kp, :, :],
                                         start=(idx == 0), stop=(idx == 7),
                                         perf_mode=DR)
                        idx += 1
                ot = outp.tile([P, O], BF16, tag="out")
                nc.vector.tensor_copy(ot[:], ps[:])
                nc.sync.dma_start(pout[mt * P:(mt + 1) * P, :], ot[:])

    nc.compile()
    return nc


def get_nc():
    global _NC
    if _NC is None:
        _NC = _build()
    return _NC


def _sigma_delta_q8(xb):
    """fp8 quantization with error feedback along t so partial sums of the
    quantization error stay O(1 ulp) instead of growing as sqrt(T)."""
    out = np.empty(xb.shape, dtype=F8NP)
    acc = np.zeros(xb.shape[1], dtype=np.float32)
    for t in range(xb.shape[0]):
        q = (xb[t] - acc).astype(F8NP)
        out[t] = q
        acc += q.astype(np.float32) - xb[t]
    return out


def prepare(x, dispatch_mask, combine_array, w1, b1, w2):
    """Host-side prep: fp8 payloads in DR layouts + coherent-channel
    corrections. Returns (in_maps, corr) where corr[b] is the rank-1
    correction to add to batch b's output."""
    w1q = w1.astype(F8NP)
    w2q = w2.astype(F8NP)
    w1qf = w1q.astype(np.float32)
    w2qf = w2q.astype(np.float32)
    dw1 = w1 - w1qf   # [E, D, HE]
    dw2 = w2 - w2qf   # [E, HE, O]

    in_maps = []
    corr = [np.zeros((T, O), dtype=np.float32) for _ in range(B)]
    sub = np.arange(0, C, C // 256)
    for m in range(8):
        b, g = m // 2, m % 2
        es = [2 * g, 2 * g + 1]
        x8 = _sigma_delta_q8(x[b])                 # [T, D] fp8
        x8f = x8.astype(np.float32)
        xb_dev = np.ascontiguousarray(
            x8.reshape(nTP, 2, P, D).transpose(0, 2, 1, 3))

        dm_dev = np.empty((2, nTP, P, 2, C), dtype=F8NP)
        cmt_dev = np.empty((2, nCP, P, 2, T), dtype=F8NP)
        bb_host = np.empty((2, HE), dtype=np.float32)
        for ei, e in enumerate(es):
            dmq = dispatch_mask[b, :, e, :].astype(F8NP)     # [T, C]
            cmq = combine_array[b, :, e, :].astype(F8NP)     # [T, C]
            dm_dev[ei] = dmq.reshape(nTP, 2, P, C).transpose(0, 2, 1, 3)
            cmt_dev[ei] = np.ascontiguousarray(cmq.T).reshape(
                nCP, 2, P, T).transpose(0, 2, 1, 3)
            dmqf = dmq.astype(np.float32)
            cmqf = cmq.astype(np.float32)
            # biasB: m = mean_c(xd_dev) computed by commuting the c-sum
            mvec = (dmqf.sum(axis=1) / C) @ x8f              # [D]
            bb_host[ei] = b1[e] + mvec @ dw1[e]
            # w2 rank-1 correction: m_h from a 256-row subsample recompute
            xd_sub = (dmqf[:, sub].T @ x8f).astype(F8NP).astype(np.float32)
            a_sub = xd_sub @ w1qf[e] + bb_host[ei][None, :]
            from scipy.special import erf
            h_sub = a_sub * 0.5 * (1.0 + erf(a_sub / np.sqrt(2.0)))
            m_h = h_sub.mean(axis=0)                          # [HE]
            corr[b] += np.outer(cmqf.sum(axis=1), m_h @ dw2[e])

        # DR layouts for weights: d(or he) = kp*256 + i*128 + p
        w1_dev = np.ascontiguousarray(
            w1q[es].reshape(2, 2, 2, P, HE).transpose(3, 0, 1, 2, 4))
        w2_dev = np.ascontiguousarray(
            w2q[es].reshape(2, 2, 2, P, O).transpose(3, 0, 1, 2, 4))
        bb_dev = np.ascontiguousarray(
            bb_host.reshape(2, nMH, P).transpose(2, 0, 1))

        in_maps.append({
            "xb": xb_dev,
            "dm": np.ascontiguousarray(dm_dev),
            "cmt": np.ascontiguousarray(cmt_dev),
            "w1": w1_dev,
            "w2": w2_dev,
            "bb": bb_dev,
        })
    return in_maps, corr


def make_in_maps(x, dispatch_mask, combine_array, w1, b1, w2):
    return prepare(x, dispatch_mask, combine_array, w1, b1, w2)[0]


def kernel(x, dispatch_mask, combine_array, w1, b1, w2, b2):
    nc = get_nc()
    x, dispatch_mask, combine_array, w1, b1, w2 = (
        np.asarray(a, dtype=np.float32)
        for a in (x, dispatch_mask, combine_array, w1, b1, w2))
    in_maps, corr = prepare(x, dispatch_mask, combine_array, w1, b1, w2)
    res = bass_utils.run_bass_kernel_spmd(nc, in_maps, core_ids=list(range(8)))
    b2f = np.asarray(b2, dtype=np.float32)
    out = np.empty((B, T, O), dtype=np.float32)
    for b in range(B):
        out[b] = (res.results[2 * b]["pout"].astype(np.float32)
                  + res.results[2 * b + 1]["pout"].astype(np.float32)
                  + corr[b] + b2f)
    return out


# revision 12
# speedup vs baseline: 1.0192x; 1.0192x over previous
"""ExpertsChooseMlp Trainium2 kernel — all-fp8 DoubleRow pipeline.

Full inputs in, full output out. Sharding: 8 cores = 4 batches x 2 expert-pairs.
Core m handles batch b=m//2 and experts {2g, 2g+1}, g=m%2. Each core computes
pout[T,O] = sum_{e in pair} combine[b,:,e,:] @ mlp_e(dispatch[b,:,e,:]^T @ x[b]);
the host sums the two partials per batch, adds b2 and a rank-1 correction.

All four matmul phases run as fp8e4m3 DoubleRow (K=256/pass, 157 TF/s):
  A: xdT[D,C] = x8^T dm8        (K=T,  64 DR passes/expert)
  B: hT[HE,C] = gelu(w18^T xdT + biasB)   (K=D,  16 passes)
  C: y[C,O]   = hT^T w28        (K=HE, 16 passes)
  D: pout[T,O]= cmT^T y         (K=C,  64 passes over expert pair)
320 matmuls/core at 216ns = ~69us PE-busy.

fp8 accuracy (meas. ~5.7e-3 max rel vs fp32 reference, gate 2e-2) relies on
three host-side corrections that cost zero device time:
 1. x is quantized with sigma-delta error feedback along t: the coherent
    channel (all-positive dispatch weights ~0.5 make the output dominated by
    t-sums of x) would otherwise amplify the sqrt(T) random walk of plain
    rounding error into ~2.4e-2.
 2. w1's quantization error rides the same coherent channel (it multiplies the
    c-mean of xd, constant across capacity slots). biasB = b1 + m @ (w1 - q8(w1))
    with m = mean_c(xd) = (rowsum(dm8)/C) @ x8, computed exactly on host.
 3. w2: same mechanism via the c-mean of h; gelu blocks commuting the mean, so
    m_h is estimated from a 256-row subsample of a host recompute, and the
    correction enters as a host-side rank-1 update R_cm (x) m_h@(w2-q8(w2)).
"""
import sys

sys.path.insert(0, "/opt/trn_rl_repo")

import numpy as np
import ml_dtypes

import concourse.bacc as bacc
import concourse.mybir as mybir
import concourse.tile as tile
from concourse import bass_utils

B, T, D, E, C, HE, O = 4, 2048, 512, 4, 1024, 512, 512
P = 128
nTP = T // 256    # 8  DR passes over T
nMD = D // P      # 4  D-chunks
nMH = HE // P     # 4  HE-chunks
nCC = C // P      # 8  C-chunks
nMT = T // P      # 16 T-chunks (phase D output)
NF = 512          # matmul free dim (one PSUM bank)

F32 = mybir.dt.float32
BF16 = mybir.dt.bfloat16
F8 = mybir.dt.float8e4
GELU = mybir.ActivationFunctionType.Gelu
DR = mybir.MatmulPerfMode.DoubleRow
nCP = nCC // 2    # 4  C pair-chunks for phase D (K=256 per matmul)

F8NP = ml_dtypes.float8_e4m3

_NC = None


def _build():
    nc = bacc.Bacc("TRN2", target_bir_lowering=False, debug=False,
                   enable_asserts=False, num_devices=1)
    # x8 in DR layout: t = tp*256 + i*128 + p  ->  xb[tp, p, i, d]
    xb = nc.dram_tensor("xb", [nTP, P, 2, D], F8, kind="ExternalInput").ap()
    # dm in DR layout per expert: dm[e, tp, p, i, c]
    dm = nc.dram_tensor("dm", [2, nTP, P, 2, C], F8, kind="ExternalInput").ap()
    # cmt planes for phase D: c = kp*256 + i*128 + p -> cmt[e, kp, p, i, t]
    cmt = nc.dram_tensor("cmt", [2, nCP, P, 2, T], F8, kind="ExternalInput").ap()
    # w1 in DR layout: d = pass*256 + i*128 + p -> w1[p, e, pass, i, he]
    w1 = nc.dram_tensor("w1", [P, 2, 2, 2, HE], F8, kind="ExternalInput").ap()
    # w2 in DR layout: he = pass*256 + i*128 + p -> w2[p, e, pass, i, o]
    w2 = nc.dram_tensor("w2", [P, 2, 2, 2, O], F8, kind="ExternalInput").ap()
    # biasB[p, e, mh] = b1 + m@(w1-q8(w1)), he = mh*128 + p
    bb = nc.dram_tensor("bb", [P, 2, nMH], F32, kind="ExternalInput").ap()
    pout = nc.dram_tensor("pout", [T, O], BF16, kind="ExternalOutput").ap()

    with tile.TileContext(nc) as tc:
        with (
            tc.tile_pool(name="const", bufs=1) as const,
            tc.tile_pool(name="dmp", bufs=16) as dmp,
            tc.tile_pool(name="cmp", bufs=8) as cmp_,
            tc.tile_pool(name="inter", bufs=1) as inter,
            tc.tile_pool(name="yp", bufs=2) as yp,
            tc.tile_pool(name="outp", bufs=4) as outp,
            tc.tile_pool(name="psum", bufs=8, space="PSUM") as psp,
        ):
            # ---- warmup memset on Vector so the GpSimd/Sync/Scalar DMA rings
            # aren't blocked behind it ----
            warm = const.tile([P, NF], BF16)
            nc.vector.memset(warm[:], 0.0)

            # ---- DMA ring assignment (each HWDGE queue sustains ~170GB/s and
            # pulls greedily; keep total early demand <= the 358GB/s HBM port
            # and keep the sync queue empty from ~45us so phase-D output
            # writes never wait):
            #   sync   : dm[e0] -> cmt[e0] -> pout writes
            #   scalar : x8 -> weights/bias -> dm[e1] -> cmt[e1]
            x_sb = const.tile([P, nTP, 2, D], F8)
            for tp in range(nTP):
                nc.scalar.dma_start(x_sb[:, tp, :, :], xb[tp])
            dm_t0 = []
            for tp in range(nTP):
                t_ = dmp.tile([P, 2, C], F8, tag="dm")
                nc.sync.dma_start(t_[:], dm[0, tp])
                dm_t0.append(t_)
            w1_sb = const.tile([P, 2, 2, 2, HE], F8)
            nc.scalar.dma_start(w1_sb[:], w1)
            w2_sb = const.tile([P, 2, 2, 2, O], F8)
            nc.scalar.dma_start(w2_sb[:], w2)
            bb_sb = const.tile([P, 2 * nMH], F32)
            nc.scalar.dma_start(bb_sb[:], bb.rearrange("p e mh -> p (e mh)"))
            dm_t1 = []
            for tp in range(nTP):
                t_ = dmp.tile([P, 2, C], F8, tag="dm")
                nc.scalar.dma_start(t_[:], dm[1, tp])
                dm_t1.append(t_)
            cmt_t = {}
            for ei in range(2):
                eng = nc.sync if ei == 0 else nc.scalar
                for kp in range(nCP):
                    t_ = cmp_.tile([P, 2, T], F8, tag="cmt")
                    eng.dma_start(t_[:], cmt[ei, kp])
                    cmt_t[(ei, kp)] = t_

            # ---- HAM warmup: dummy matmuls on zeroed SBUF during the initial
            # DMA wait so real matmuls start at 2.4GHz. ----
            ps_w = psp.tile([P, NF], F32, tag="ps", name="ps_warm")
            for i in range(8):
                nc.tensor.matmul(ps_w[:], warm[:, 0:P], warm[:],
                                 start=(i == 0), stop=(i == 7))

            y_tiles = []
            for ei in range(2):
                dm_t = dm_t0 if ei == 0 else dm_t1

                # ---- phase A: xdT[D, C] = x8^T dm8 (fp8 DR) ----
                # tp-outer: all 8 PSUM banks accumulate in parallel, each dm
                # tile consumed once and released for the next expert prefetch.
                xdt = inter.tile([P, nMD, C], F8, tag="xdt")
                pss = [psp.tile([P, NF], F32, tag="ps", name=f"psa{i}")
                       for i in range(2 * nMD)]
                for tp in range(nTP):
                    for mc in range(nMD):
                        lhsT = x_sb[:, tp, :, mc * P:(mc + 1) * P]
                        nc.tensor.matmul(pss[2 * mc][:], lhsT,
                                         dm_t[tp][:, :, 0:NF],
                                         start=(tp == 0), stop=(tp == nTP - 1),
                                         perf_mode=DR)
                        nc.tensor.matmul(pss[2 * mc + 1][:], lhsT,
                                         dm_t[tp][:, :, NF:C],
                                         start=(tp == 0), stop=(tp == nTP - 1),
                                         perf_mode=DR)
                for ncc in range(2):
                    for mc in range(nMD):
                        nc.vector.tensor_copy(xdt[:, mc, ncc * NF:(ncc + 1) * NF],
                                              pss[2 * mc + ncc][:])

                # ---- phase B: hT[HE, C] = gelu(w18^T xdT + biasB) (fp8 DR) ----
                # ncc-outer so phase C's first C-half unblocks early.
                ht = inter.tile([P, nMH, C], F8, tag="ht")
                for ncc in range(2):
                    sl = slice(ncc * NF, (ncc + 1) * NF)
                    for mh in range(nMH):
                        ps0 = psp.tile([P, NF], F32, tag="ps")
                        for kp in range(2):
                            nc.tensor.matmul(
                                ps0[:],
                                w1_sb[:, ei, kp, :, mh * P:(mh + 1) * P],
                                xdt[:, 2 * kp:2 * kp + 2, sl],
                                start=(kp == 0), stop=(kp == 1), perf_mode=DR)
                        bia = bb_sb[:, ei * nMH + mh:ei * nMH + mh + 1]
                        nc.scalar.activation(ht[:, mh, sl], ps0[:], GELU, bias=bia)

                # ---- phase C: y[C, O] = hT^T w28 (fp8 DR), stored in DR plane
                # layout for phase D: row c = kp*256 + i*128 + p -> y_sb[p,kp,i,:]
                y_sb = yp.tile([P, nCP, 2, O], F8, tag="y")
                for cc in range(nCC):
                    ps = psp.tile([P, NF], F32, tag="ps")
                    for kp in range(2):
                        nc.tensor.matmul(ps[:],
                                         ht[:, 2 * kp:2 * kp + 2, cc * P:(cc + 1) * P],
                                         w2_sb[:, ei, kp, :, :],
                                         start=(kp == 0), stop=(kp == 1),
                                         perf_mode=DR)
                    nc.vector.tensor_copy(y_sb[:, cc // 2, cc % 2, :], ps[:])
                y_tiles.append(y_sb)

            # ---- phase D: pout[T, O] = sum_e cmT_e^T y_e (fp8 DR) ----
            for mt in range(nMT):
                ps = psp.tile([P, NF], F32, tag="ps")
                idx = 0
                for ei in range(2):
                    for kp in range(nCP):
                        nc.tensor.matmul(ps[:],
                                         cmt_t[(ei, kp)][:, :, mt * P:(mt + 1) * P],
                                         y_tiles[ei][:, kp, :, :],
                                         start=(idx == 0), stop=(idx == 7),
                                         perf_mode=DR)
                        idx += 1
                ot = outp.tile([P, O], BF16, tag="out")
                nc.vector.tensor_copy(ot[:], ps[:])
                nc.sync.dma_start(pout[mt * P:(mt + 1) * P, :], ot[:])

    nc.compile()
    return nc


def get_nc():
    global _NC
    if _NC is None:
        _NC = _build()
    return _NC


def _sigma_delta_q8(xb):
    """fp8 quantization with error feedback along t so partial sums of the
    quantization error stay O(1 ulp) instead of growing as sqrt(T)."""
    out = np.empty(xb.shape, dtype=F8NP)
    acc = np.zeros(xb.shape[1], dtype=np.float32)
    for t in range(xb.shape[0]):
        q = (xb[t] - acc).astype(F8NP)
        out[t] = q
        acc += q.astype(np.float32) - xb[t]
    return out


def prepare(x, dispatch_mask, combine_array, w1, b1, w2):
    """Host-side prep: fp8 payloads in DR layouts + coherent-channel
    corrections. Returns (in_maps, corr) where corr[b] is the rank-1
    correction to add to batch b's output."""
    w1q = w1.astype(F8NP)
    w2q = w2.astype(F8NP)
    w1qf = w1q.astype(np.float32)
    w2qf = w2q.astype(np.float32)
    dw1 = w1 - w1qf   # [E, D, HE]
    dw2 = w2 - w2qf   # [E, HE, O]

    in_maps = []
    corr = [np.zeros((T, O), dtype=np.float32) for _ in range(B)]
    sub = np.arange(0, C, C // 256)
    for m in range(8):
        b, g = m // 2, m % 2
        es = [2 * g, 2 * g + 1]
        x8 = _sigma_delta_q8(x[b])                 # [T, D] fp8
        x8f = x8.astype(np.float32)
        xb_dev = np.ascontiguousarray(
            x8.reshape(nTP, 2, P, D).transpose(0, 2, 1, 3))

        dm_dev = np.empty((2, nTP, P, 2, C), dtype=F8NP)
        cmt_dev = np.empty((2, nCP, P, 2, T), dtype=F8NP)
        bb_host = np.empty((2, HE), dtype=np.float32)
        for ei, e in enumerate(es):
            dmq = dispatch_mask[b, :, e, :].astype(F8NP)     # [T, C]
            cmq = combine_array[b, :, e, :].astype(F8NP)     # [T, C]
            dm_dev[ei] = dmq.reshape(nTP, 2, P, C).transpose(0, 2, 1, 3)
            cmt_dev[ei] = np.ascontiguousarray(cmq.T).reshape(
                nCP, 2, P, T).transpose(0, 2, 1, 3)
            dmqf = dmq.astype(np.float32)
            cmqf = cmq.astype(np.float32)
            # biasB: m = mean_c(xd_dev) computed by commuting the c-sum
            mvec = (dmqf.sum(axis=1) / C) @ x8f              # [D]
            bb_host[ei] = b1[e] + mvec @ dw1[e]
            # w2 rank-1 correction: m_h from a 256-row subsample recompute
            xd_sub = (dmqf[:, sub].T @ x8f).astype(F8NP).astype(np.float32)
            a_sub = xd_sub @ w1qf[e] + bb_host[ei][None, :]
            from scipy.special import erf
            h_sub = a_sub * 0.5 * (1.0 + erf(a_sub / np.sqrt(2.0)))
            m_h = h_sub.mean(axis=0)                          # [HE]
            corr[b] += np.outer(cmqf.sum(axis=1), m_h @ dw2[e])

        # DR layouts for weights: d(or he) = kp*256 + i*128 + p
        w1_dev = np.ascontiguousarray(
            w1q[es].reshape(2, 2, 2, P, HE).transpose(3, 0, 1, 2, 4))
        w2_dev = np.ascontiguousarray(
            w2q[es].reshape(2, 2, 2, P, O).transpose(3, 0, 1, 2, 4))
        bb_dev = np.ascontiguousarray(
            bb_host.reshape(2, nMH, P).transpose(2, 0, 1))

        in_maps.append({
            "xb": xb_dev,
            "dm": np.ascontiguousarray(dm_dev),
            "cmt": np.ascontiguousarray(cmt_dev),
            "w1": w1_dev,
            "w2": w2_dev,
            "bb": bb_dev,
        })
    return in_maps, corr


def make_in_maps(x, dispatch_mask, combine_array, w1, b1, w2):
    return prepare(x, dispatch_mask, combine_array, w1, b1, w2)[0]


def kernel(x, dispatch_mask, combine_array, w1, b1, w2, b2):
    nc = get_nc()
    x, dispatch_mask, combine_array, w1, b1, w2 = (
        np.asarray(a, dtype=np.float32)
        for a in (x, dispatch_mask, combine_array, w1, b1, w2))
    in_maps, corr = prepare(x, dispatch_mask, combine_array, w1, b1, w2)
    res = bass_utils.run_bass_kernel_spmd(nc, in_maps, core_ids=list(range(8)))
    b2f = np.asarray(b2, dtype=np.float32)
    out = np.empty((B, T, O), dtype=np.float32)
    for b in range(B):
        out[b] = (res.results[2 * b]["pout"].astype(np.float32)
                  + res.results[2 * b + 1]["pout"].astype(np.float32)
                  + corr[b] + b2f)
    return out


# revision 15
# speedup vs baseline: 1.1328x; 1.1114x over previous
"""ExpertsChooseMlp Trainium2 kernel — all-fp8 DoubleRow pipeline.

Full inputs in, full output out. Sharding: 8 cores = 4 batches x 2 expert-pairs.
Core m handles batch b=m//2 and experts {2g, 2g+1}, g=m%2. Each core computes
pout[T,O] = sum_{e in pair} combine[b,:,e,:] @ mlp_e(dispatch[b,:,e,:]^T @ x[b]);
the host sums the two partials per batch, adds b2 and a rank-1 correction.

All four matmul phases run as fp8e4m3 DoubleRow (K=256/pass, 157 TF/s):
  A: xdT[D,C] = x8^T dm8        (K=T,  64 DR passes/expert)
  B: hT[HE,C] = gelu(w18^T xdT + biasB)   (K=D,  16 passes)
  C: y[C,O]   = hT^T w28        (K=HE, 16 passes)
  D: pout[T,O]= cmT^T y         (K=C,  64 passes over expert pair)
320 matmuls/core at 216ns = ~69us PE-busy.

fp8 accuracy (meas. ~5.7e-3 max rel vs fp32 reference, gate 2e-2) relies on
three host-side corrections that cost zero device time:
 1. x is quantized with sigma-delta error feedback along t: the coherent
    channel (all-positive dispatch weights ~0.5 make the output dominated by
    t-sums of x) would otherwise amplify the sqrt(T) random walk of plain
    rounding error into ~2.4e-2.
 2. w1's quantization error rides the same coherent channel (it multiplies the
    c-mean of xd, constant across capacity slots). biasB = b1 + m @ (w1 - q8(w1))
    with m = mean_c(xd) = (rowsum(dm8)/C) @ x8, computed exactly on host.
 3. w2: same mechanism via the c-mean of h; gelu blocks commuting the mean, so
    m_h is estimated from a 256-row subsample of a host recompute, and the
    correction enters as a host-side rank-1 update R_cm (x) m_h@(w2-q8(w2)).
"""
import sys

sys.path.insert(0, "/opt/trn_rl_repo")

import numpy as np
import ml_dtypes

import concourse.bacc as bacc
import concourse.mybir as mybir
import concourse.tile as tile
from concourse import bass_utils

B, T, D, E, C, HE, O = 4, 2048, 512, 4, 1024, 512, 512
P = 128
nTP = T // 256    # 8  DR passes over T
nMD = D // P      # 4  D-chunks
nMH = HE // P     # 4  HE-chunks
nCC = C // P      # 8  C-chunks
nMT = T // P      # 16 T-chunks (phase D output)
NF = 512          # matmul free dim (one PSUM bank)

F32 = mybir.dt.float32
BF16 = mybir.dt.bfloat16
F8 = mybir.dt.float8e4
GELU = mybir.ActivationFunctionType.Gelu
DR = mybir.MatmulPerfMode.DoubleRow
nCP = nCC // 2    # 4  C pair-chunks for phase D (K=256 per matmul)

F8NP = ml_dtypes.float8_e4m3

_NC = None


def _build():
    nc = bacc.Bacc("TRN2", target_bir_lowering=False, debug=False,
                   enable_asserts=False, num_devices=1)
    # x8 in DR layout: t = tp*256 + i*128 + p  ->  xb[tp, p, i, d]
    xb = nc.dram_tensor("xb", [nTP, P, 2, D], F8, kind="ExternalInput").ap()
    # dm in DR layout per expert: dm[e, tp, p, i, c]
    dm = nc.dram_tensor("dm", [2, nTP, P, 2, C], F8, kind="ExternalInput").ap()
    # cmt planes for phase D: c = kp*256 + i*128 + p -> cmt[e, kp, p, i, t]
    cmt = nc.dram_tensor("cmt", [2, nCP, P, 2, T], F8, kind="ExternalInput").ap()
    # w1 in DR layout: d = pass*256 + i*128 + p -> w1[p, e, pass, i, he]
    w1 = nc.dram_tensor("w1", [P, 2, 2, 2, HE], F8, kind="ExternalInput").ap()
    # w2 in DR layout: he = pass*256 + i*128 + p -> w2[p, e, pass, i, o]
    w2 = nc.dram_tensor("w2", [P, 2, 2, 2, O], F8, kind="ExternalInput").ap()
    # biasB[p, e, mh] = b1 + m@(w1-q8(w1)), he = mh*128 + p
    bb = nc.dram_tensor("bb", [P, 2, nMH], F32, kind="ExternalInput").ap()
    pout = nc.dram_tensor("pout", [T, O], BF16, kind="ExternalOutput").ap()

    with tile.TileContext(nc) as tc:
        with (
            tc.tile_pool(name="const", bufs=1) as const,
            tc.tile_pool(name="dmp", bufs=16) as dmp,
            tc.tile_pool(name="cmp", bufs=8) as cmp_,
            tc.tile_pool(name="inter", bufs=1) as inter,
            tc.tile_pool(name="yp", bufs=2) as yp,
            tc.tile_pool(name="outp", bufs=4) as outp,
            tc.tile_pool(name="psum", bufs=8, space="PSUM") as psp,
        ):
            # ---- warmup memset on Vector so the GpSimd/Sync/Scalar DMA rings
            # aren't blocked behind it ----
            warm = const.tile([P, NF], BF16)
            nc.vector.memset(warm[:], 0.0)

            # ---- DMA ring assignment. Queues pull greedily and contend for
            # the ~358GB/s HBM port, so later-needed bulk must sit BEHIND
            # earlier-needed bulk on the same queue (FIFO paces it), not on a
            # parallel queue where it would starve the critical stream:
            #   sync   : dm[e0] -> dm[e1] -> cmt[e0] -> cmt[e1]  (need-order)
            #   scalar : x8 -> weights/bias -> (idle) -> pout writes
            x_sb = const.tile([P, nTP, 2, D], F8)
            for tp in range(nTP):
                nc.scalar.dma_start(x_sb[:, tp, :, :], xb[tp])
            dm_t0 = []
            for tp in range(nTP):
                t_ = dmp.tile([P, 2, C], F8, tag="dm")
                nc.sync.dma_start(t_[:], dm[0, tp])
                dm_t0.append(t_)
            w1_sb = const.tile([P, 2, 2, 2, HE], F8)
            nc.scalar.dma_start(w1_sb[:], w1)
            w2_sb = const.tile([P, 2, 2, 2, O], F8)
            nc.scalar.dma_start(w2_sb[:], w2)
            bb_sb = const.tile([P, 2 * nMH], F32)
            nc.scalar.dma_start(bb_sb[:], bb.rearrange("p e mh -> p (e mh)"))
            dm_t1 = []
            for tp in range(nTP):
                t_ = dmp.tile([P, 2, C], F8, tag="dm")
                nc.sync.dma_start(t_[:], dm[1, tp])
                dm_t1.append(t_)
            cmt_t = {}
            for ei in range(2):
                for kp in range(nCP):
                    t_ = cmp_.tile([P, 2, T], F8, tag="cmt")
                    nc.sync.dma_start(t_[:], cmt[ei, kp])
                    cmt_t[(ei, kp)] = t_

            # ---- HAM warmup: dummy matmuls on zeroed SBUF during the initial
            # DMA wait so real matmuls start at 2.4GHz. ----
            ps_w = psp.tile([P, NF], F32, tag="ps", name="ps_warm")
            for i in range(8):
                nc.tensor.matmul(ps_w[:], warm[:, 0:P], warm[:],
                                 start=(i == 0), stop=(i == 7))

            y_tiles = []
            for ei in range(2):
                dm_t = dm_t0 if ei == 0 else dm_t1

                # ---- phase A: xdT[D, C] = x8^T dm8 (fp8 DR) ----
                # tp-outer: all 8 PSUM banks accumulate in parallel, each dm
                # tile consumed once and released for the next expert prefetch.
                xdt = inter.tile([P, nMD, C], F8, tag="xdt")
                pss = [psp.tile([P, NF], F32, tag="ps", name=f"psa{i}")
                       for i in range(2 * nMD)]
                for tp in range(nTP):
                    for mc in range(nMD):
                        lhsT = x_sb[:, tp, :, mc * P:(mc + 1) * P]
                        nc.tensor.matmul(pss[2 * mc][:], lhsT,
                                         dm_t[tp][:, :, 0:NF],
                                         start=(tp == 0), stop=(tp == nTP - 1),
                                         perf_mode=DR)
                        nc.tensor.matmul(pss[2 * mc + 1][:], lhsT,
                                         dm_t[tp][:, :, NF:C],
                                         start=(tp == 0), stop=(tp == nTP - 1),
                                         perf_mode=DR)
                for ncc in range(2):
                    for mc in range(nMD):
                        nc.vector.tensor_copy(xdt[:, mc, ncc * NF:(ncc + 1) * NF],
                                              pss[2 * mc + ncc][:])

                # ---- phase B: hT[HE, C] = gelu(w18^T xdT + biasB) (fp8 DR) ----
                # ncc-outer so phase C's first C-half unblocks early.
                ht = inter.tile([P, nMH, C], F8, tag="ht")
                for ncc in range(2):
                    sl = slice(ncc * NF, (ncc + 1) * NF)
                    for mh in range(nMH):
                        ps0 = psp.tile([P, NF], F32, tag="ps")
                        for kp in range(2):
                            nc.tensor.matmul(
                                ps0[:],
                                w1_sb[:, ei, kp, :, mh * P:(mh + 1) * P],
                                xdt[:, 2 * kp:2 * kp + 2, sl],
                                start=(kp == 0), stop=(kp == 1), perf_mode=DR)
                        bia = bb_sb[:, ei * nMH + mh:ei * nMH + mh + 1]
                        nc.scalar.activation(ht[:, mh, sl], ps0[:], GELU, bias=bia)

                # ---- phase C: y[C, O] = hT^T w28 (fp8 DR), stored in DR plane
                # layout for phase D: row c = kp*256 + i*128 + p -> y_sb[p,kp,i,:]
                y_sb = yp.tile([P, nCP, 2, O], F8, tag="y")
                for cc in range(nCC):
                    ps = psp.tile([P, NF], F32, tag="ps")
                    for kp in range(2):
                        nc.tensor.matmul(ps[:],
                                         ht[:, 2 * kp:2 * kp + 2, cc * P:(cc + 1) * P],
                                         w2_sb[:, ei, kp, :, :],
                                         start=(kp == 0), stop=(kp == 1),
                                         perf_mode=DR)
                    nc.vector.tensor_copy(y_sb[:, cc // 2, cc % 2, :], ps[:])
                y_tiles.append(y_sb)

            # ---- phase D: pout[T, O] = sum_e cmT_e^T y_e (fp8 DR) ----
            for mt in range(nMT):
                ps = psp.tile([P, NF], F32, tag="ps")
                idx = 0
                for ei in range(2):
                    for kp in range(nCP):
                        nc.tensor.matmul(ps[:],
                                         cmt_t[(ei, kp)][:, :, mt * P:(mt + 1) * P],
                                         y_tiles[ei][:, kp, :, :],
                                         start=(idx == 0), stop=(idx == 7),
                                         perf_mode=DR)
                        idx += 1
                ot = outp.tile([P, O], BF16, tag="out")
                nc.vector.tensor_copy(ot[:], ps[:])
                nc.scalar.dma_start(pout[mt * P:(mt + 1) * P, :], ot[:])

    nc.compile()
    return nc


def get_nc():
    global _NC
    if _NC is None:
        _NC = _build()
    return _NC


def _sigma_delta_q8(xb):
    """fp8 quantization with error feedback along t so partial sums of the
    quantization error stay O(1 ulp) instead of growing as sqrt(T)."""
    out = np.empty(xb.shape, dtype=F8NP)
    acc = np.zeros(xb.shape[1], dtype=np.float32)
    for t in range(xb.shape[0]):
        q = (xb[t] - acc).astype(F8NP)
        out[t] = q
        acc += q.astype(np.float32) - xb[t]
    return out


def prepare(x, dispatch_mask, combine_array, w1, b1, w2):
    """Host-side prep: fp8 payloads in DR layouts + coherent-channel
    corrections. Returns (in_maps, corr) where corr[b] is the rank-1
    correction to add to batch b's output."""
    w1q = w1.astype(F8NP)
    w2q = w2.astype(F8NP)
    w1qf = w1q.astype(np.float32)
    w2qf = w2q.astype(np.float32)
    dw1 = w1 - w1qf   # [E, D, HE]
    dw2 = w2 - w2qf   # [E, HE, O]

    in_maps = []
    corr = [np.zeros((T, O), dtype=np.float32) for _ in range(B)]
    sub = np.arange(0, C, C // 256)
    for m in range(8):
        b, g = m // 2, m % 2
        es = [2 * g, 2 * g + 1]
        x8 = _sigma_delta_q8(x[b])                 # [T, D] fp8
        x8f = x8.astype(np.float32)
        xb_dev = np.ascontiguousarray(
            x8.reshape(nTP, 2, P, D).transpose(0, 2, 1, 3))

        dm_dev = np.empty((2, nTP, P, 2, C), dtype=F8NP)
        cmt_dev = np.empty((2, nCP, P, 2, T), dtype=F8NP)
        bb_host = np.empty((2, HE), dtype=np.float32)
        for ei, e in enumerate(es):
            dmq = dispatch_mask[b, :, e, :].astype(F8NP)     # [T, C]
            cmq = combine_array[b, :, e, :].astype(F8NP)     # [T, C]
            dm_dev[ei] = dmq.reshape(nTP, 2, P, C).transpose(0, 2, 1, 3)
            cmt_dev[ei] = np.ascontiguousarray(cmq.T).reshape(
                nCP, 2, P, T).transpose(0, 2, 1, 3)
            dmqf = dmq.astype(np.float32)
            cmqf = cmq.astype(np.float32)
            # biasB: m = mean_c(xd_dev) computed by commuting the c-sum
            mvec = (dmqf.sum(axis=1) / C) @ x8f              # [D]
            bb_host[ei] = b1[e] + mvec @ dw1[e]
            # w2 rank-1 correction: m_h from a 256-row subsample recompute
            xd_sub = (dmqf[:, sub].T @ x8f).astype(F8NP).astype(np.float32)
            a_sub = xd_sub @ w1qf[e] + bb_host[ei][None, :]
            from scipy.special import erf
            h_sub = a_sub * 0.5 * (1.0 + erf(a_sub / np.sqrt(2.0)))
            m_h = h_sub.mean(axis=0)                          # [HE]
            corr[b] += np.outer(cmqf.sum(axis=1), m_h @ dw2[e])

        # DR layouts for weights: d(or he) = kp*256 + i*128 + p
        w1_dev = np.ascontiguousarray(
            w1q[es].reshape(2, 2, 2, P, HE).transpose(3, 0, 1, 2, 4))
        w2_dev = np.ascontiguousarray(
            w2q[es].reshape(2, 2, 2, P, O).transpose(3, 0, 1, 2, 4))
        bb_dev = np.ascontiguousarray(
            bb_host.reshape(2, nMH, P).transpose(2, 0, 1))

        in_maps.append({
            "xb": xb_dev,
            "dm": np.ascontiguousarray(dm_dev),
            "cmt": np.ascontiguousarray(cmt_dev),
            "w1": w1_dev,
            "w2": w2_dev,
            "bb": bb_dev,
        })
    return in_maps, corr


def make_in_maps(x, dispatch_mask, combine_array, w1, b1, w2):
    return prepare(x, dispatch_mask, combine_array, w1, b1, w2)[0]


def kernel(x, dispatch_mask, combine_array, w1, b1, w2, b2):
    nc = get_nc()
    x, dispatch_mask, combine_array, w1, b1, w2 = (
        np.asarray(a, dtype=np.float32)
        for a in (x, dispatch_mask, combine_array, w1, b1, w2))
    in_maps, corr = prepare(x, dispatch_mask, combine_array, w1, b1, w2)
    res = bass_utils.run_bass_kernel_spmd(nc, in_maps, core_ids=list(range(8)))
    b2f = np.asarray(b2, dtype=np.float32)
    out = np.empty((B, T, O), dtype=np.float32)
    for b in range(B):
        out[b] = (res.results[2 * b]["pout"].astype(np.float32)
                  + res.results[2 * b + 1]["pout"].astype(np.float32)
                  + corr[b] + b2f)
    return out


# revision 21
# speedup vs baseline: 1.1423x; 1.0083x over previous
"""ExpertsChooseMlp Trainium2 kernel — all-fp8 DoubleRow pipeline.

Full inputs in, full output out. Sharding: 8 cores = 4 batches x 2 expert-pairs.
Core m handles batch b=m//2 and experts {2g, 2g+1}, g=m%2. Each core computes
pout[T,O] = sum_{e in pair} combine[b,:,e,:] @ mlp_e(dispatch[b,:,e,:]^T @ x[b]);
the host sums the two partials per batch, adds b2 and a rank-1 correction.

All four matmul phases run as fp8e4m3 DoubleRow (K=256/pass, 157 TF/s):
  A: xdT[D,C] = x8^T dm8        (K=T,  64 DR passes/expert)
  B: hT[HE,C] = gelu(w18^T xdT + biasB)   (K=D,  16 passes)
  C: y[C,O]   = hT^T w28        (K=HE, 16 passes)
  D: pout[T,O]= cmT^T y         (K=C,  64 passes over expert pair)
320 matmuls/core at 216ns = ~69us PE-busy.

fp8 accuracy (meas. ~5.7e-3 max rel vs fp32 reference, gate 2e-2) relies on
three host-side corrections that cost zero device time:
 1. x is quantized with sigma-delta error feedback along t: the coherent
    channel (all-positive dispatch weights ~0.5 make the output dominated by
    t-sums of x) would otherwise amplify the sqrt(T) random walk of plain
    rounding error into ~2.4e-2.
 2. w1's quantization error rides the same coherent channel (it multiplies the
    c-mean of xd, constant across capacity slots). biasB = b1 + m @ (w1 - q8(w1))
    with m = mean_c(xd) = (rowsum(dm8)/C) @ x8, computed exactly on host.
 3. w2: same mechanism via the c-mean of h; gelu blocks commuting the mean, so
    m_h is estimated from a 256-row subsample of a host recompute, and the
    correction enters as a host-side rank-1 update R_cm (x) m_h@(w2-q8(w2)).
"""
import sys

sys.path.insert(0, "/opt/trn_rl_repo")

import numpy as np
import ml_dtypes

import concourse.bacc as bacc
import concourse.mybir as mybir
import concourse.tile as tile
from concourse import bass_utils

B, T, D, E, C, HE, O = 4, 2048, 512, 4, 1024, 512, 512
P = 128
nTP = T // 256    # 8  DR passes over T
nMD = D // P      # 4  D-chunks
nMH = HE // P     # 4  HE-chunks
nCC = C // P      # 8  C-chunks
nMT = T // P      # 16 T-chunks (phase D output)
NF = 512          # matmul free dim (one PSUM bank)

F32 = mybir.dt.float32
BF16 = mybir.dt.bfloat16
F8 = mybir.dt.float8e4
GELU = mybir.ActivationFunctionType.Gelu
COPY = mybir.ActivationFunctionType.Copy
DR = mybir.MatmulPerfMode.DoubleRow
nCP = nCC // 2    # 4  C pair-chunks for phase D (K=256 per matmul)

F8NP = ml_dtypes.float8_e4m3

_NC = None


def _build():
    nc = bacc.Bacc("TRN2", target_bir_lowering=False, debug=False,
                   enable_asserts=False, num_devices=1)
    # x8 in DR layout: t = tp*256 + i*128 + p  ->  xb[tp, p, i, d]
    xb = nc.dram_tensor("xb", [nTP, P, 2, D], F8, kind="ExternalInput").ap()
    # dm in DR layout per expert: dm[e, tp, p, i, c]
    dm = nc.dram_tensor("dm", [2, nTP, P, 2, C], F8, kind="ExternalInput").ap()
    # cmt planes for phase D: c = kp*256 + i*128 + p -> cmt[e, kp, p, i, t]
    cmt = nc.dram_tensor("cmt", [2, nCP, P, 2, T], F8, kind="ExternalInput").ap()
    # w1 in DR layout: d = pass*256 + i*128 + p -> w1[p, e, pass, i, he]
    w1 = nc.dram_tensor("w1", [P, 2, 2, 2, HE], F8, kind="ExternalInput").ap()
    # w2 in DR layout: he = pass*256 + i*128 + p -> w2[p, e, pass, i, o]
    w2 = nc.dram_tensor("w2", [P, 2, 2, 2, O], F8, kind="ExternalInput").ap()
    # biasB[p, e, mh] = b1 + m@(w1-q8(w1)), he = mh*128 + p
    bb = nc.dram_tensor("bb", [P, 2, nMH], F32, kind="ExternalInput").ap()
    pout = nc.dram_tensor("pout", [T, O], BF16, kind="ExternalOutput").ap()

    with tile.TileContext(nc) as tc:
        with (
            tc.tile_pool(name="const", bufs=1) as const,
            tc.tile_pool(name="dmp", bufs=16) as dmp,
            tc.tile_pool(name="cmp", bufs=8) as cmp_,
            tc.tile_pool(name="inter", bufs=1) as inter,
            tc.tile_pool(name="yp", bufs=2) as yp,
            tc.tile_pool(name="outp", bufs=4) as outp,
            tc.tile_pool(name="psum", bufs=8, space="PSUM") as psp,
        ):
            # ---- warmup memset on Vector so the GpSimd/Sync/Scalar DMA rings
            # aren't blocked behind it ----
            warm = const.tile([P, NF], BF16)
            nc.vector.memset(warm[:], 0.0)
            # preload the gelu activation table now (it otherwise loads lazily
            # at the first phase-B ACTIVATE, stalling it ~1.3us)
            warm8 = const.tile([P, 1], F8)
            nc.scalar.activation(warm8[:], warm[:, 0:1], GELU)

            # ---- DMA ring assignment. Queues pull greedily and contend for
            # the ~358GB/s HBM port, so later-needed bulk must sit BEHIND
            # earlier-needed bulk on the same queue (FIFO paces it), not on a
            # parallel queue where it would starve the critical stream:
            #   sync   : dm[e0] -> dm[e1] -> cmt[e0] -> cmt[e1]  (need-order)
            #   scalar : x8 -> weights/bias -> (idle) -> pout writes
            x_sb = const.tile([P, nTP, 2, D], F8)
            for tp in range(nTP):
                nc.scalar.dma_start(x_sb[:, tp, :, :], xb[tp])
            dm_t0 = []
            for tp in range(nTP):
                t_ = dmp.tile([P, 2, C], F8, tag="dm")
                nc.sync.dma_start(t_[:], dm[0, tp])
                dm_t0.append(t_)
            w1_sb = const.tile([P, 2, 2, 2, HE], F8)
            nc.scalar.dma_start(w1_sb[:], w1)
            w2_sb = const.tile([P, 2, 2, 2, O], F8)
            nc.scalar.dma_start(w2_sb[:], w2)
            bb_sb = const.tile([P, 2 * nMH], F32)
            nc.scalar.dma_start(bb_sb[:], bb.rearrange("p e mh -> p (e mh)"))
            dm_t1 = []
            for tp in range(nTP):
                t_ = dmp.tile([P, 2, C], F8, tag="dm")
                nc.sync.dma_start(t_[:], dm[1, tp])
                dm_t1.append(t_)
            cmt_t = {}
            for ei in range(2):
                for kp in range(nCP):
                    t_ = cmp_.tile([P, 2, T], F8, tag="cmt")
                    nc.sync.dma_start(t_[:], cmt[ei, kp])
                    cmt_t[(ei, kp)] = t_

            # ---- HAM warmup: dummy matmuls on zeroed SBUF during the initial
            # DMA wait so real matmuls start at 2.4GHz. ----
            ps_w = psp.tile([P, NF], F32, tag="ps", name="ps_warm")
            for i in range(8):
                nc.tensor.matmul(ps_w[:], warm[:, 0:P], warm[:],
                                 start=(i == 0), stop=(i == 7))

            y_tiles = []
            for ei in range(2):
                dm_t = dm_t0 if ei == 0 else dm_t1

                # ---- phase A: xdT[D, C] = x8^T dm8 (fp8 DR) ----
                # tp-outer: all 8 PSUM banks accumulate in parallel, each dm
                # tile consumed once and released for the next expert prefetch.
                # The final pass runs bank-by-bank (ncc0 banks first) with the
                # drain issued right behind each stop, split across Vector and
                # GpSimd, so phase B starts ~0.5us after A's last matmul
                # instead of waiting for 8 serialized casts.
                xdt = inter.tile([P, nMD, C], F8, tag="xdt")
                pss = [psp.tile([P, NF], F32, tag="ps", name=f"psa{i}")
                       for i in range(2 * nMD)]
                for tp in range(nTP - 1):
                    for mc in range(nMD):
                        lhsT = x_sb[:, tp, :, mc * P:(mc + 1) * P]
                        nc.tensor.matmul(pss[2 * mc][:], lhsT,
                                         dm_t[tp][:, :, 0:NF],
                                         start=(tp == 0), stop=False,
                                         perf_mode=DR)
                        nc.tensor.matmul(pss[2 * mc + 1][:], lhsT,
                                         dm_t[tp][:, :, NF:C],
                                         start=(tp == 0), stop=False,
                                         perf_mode=DR)
                for ncc in range(2):
                    for mc in range(nMD):
                        nc.tensor.matmul(
                            pss[2 * mc + ncc][:],
                            x_sb[:, nTP - 1, :, mc * P:(mc + 1) * P],
                            dm_t[nTP - 1][:, :, ncc * NF:(ncc + 1) * NF],
                            start=False, stop=True, perf_mode=DR)
                        # split drains: Vector casts ncc0, Scalar copies ncc1
                        # (GpSimd cannot read PSUM)
                        if ncc == 0:
                            nc.vector.tensor_copy(
                                xdt[:, mc, 0:NF], pss[2 * mc][:])
                        else:
                            nc.scalar.activation(
                                xdt[:, mc, NF:C], pss[2 * mc + 1][:], COPY)

                # ---- phase B: hT[HE, C] = gelu(w18^T xdT + biasB) (fp8 DR) ----
                # ncc-outer; within an ncc, kp-outer across 4 banks so the
                # first 4 matmuls only need xdt chunks 0-1.
                ht = inter.tile([P, nMH, C], F8, tag="ht")
                for ncc in range(2):
                    sl = slice(ncc * NF, (ncc + 1) * NF)
                    psB = [psp.tile([P, NF], F32, tag="ps",
                                    name=f"psb{ei}{ncc}{i}") for i in range(nMH)]
                    for kp in range(2):
                        for mh in range(nMH):
                            nc.tensor.matmul(
                                psB[mh][:],
                                w1_sb[:, ei, kp, :, mh * P:(mh + 1) * P],
                                xdt[:, 2 * kp:2 * kp + 2, sl],
                                start=(kp == 0), stop=(kp == 1), perf_mode=DR)
                    for mh in range(nMH):
                        bia = bb_sb[:, ei * nMH + mh:ei * nMH + mh + 1]
                        nc.scalar.activation(ht[:, mh, sl], psB[mh][:], GELU,
                                             bias=bia)

                # ---- phase C: y[C, O] = hT^T w28 (fp8 DR), stored in DR plane
                # layout for phase D: row c = kp*256 + i*128 + p -> y_sb[p,kp,i,:]
                y_sb = yp.tile([P, nCP, 2, O], F8, tag="y")
                for cc in range(nCC):
                    ps = psp.tile([P, NF], F32, tag="ps")
                    for kp in range(2):
                        nc.tensor.matmul(ps[:],
                                         ht[:, 2 * kp:2 * kp + 2, cc * P:(cc + 1) * P],
                                         w2_sb[:, ei, kp, :, :],
                                         start=(kp == 0), stop=(kp == 1),
                                         perf_mode=DR)
                    if cc % 2 == 0:
                        nc.vector.tensor_copy(y_sb[:, cc // 2, 0, :], ps[:])
                    else:
                        nc.scalar.activation(y_sb[:, cc // 2, 1, :], ps[:], COPY)
                y_tiles.append(y_sb)

            # ---- phase D: pout[T, O] = sum_e cmT_e^T y_e (fp8 DR) ----
            for mt in range(nMT):
                ps = psp.tile([P, NF], F32, tag="ps")
                idx = 0
                for ei in range(2):
                    for kp in range(nCP):
                        nc.tensor.matmul(ps[:],
                                         cmt_t[(ei, kp)][:, :, mt * P:(mt + 1) * P],
                                         y_tiles[ei][:, kp, :, :],
                                         start=(idx == 0), stop=(idx == 7),
                                         perf_mode=DR)
                        idx += 1
                ot = outp.tile([P, O], BF16, tag="out")
                nc.vector.tensor_copy(ot[:], ps[:])
                nc.scalar.dma_start(pout[mt * P:(mt + 1) * P, :], ot[:])

    nc.compile()
    return nc


def get_nc():
    global _NC
    if _NC is None:
        _NC = _build()
    return _NC


def _sigma_delta_q8(xb):
    """fp8 quantization with error feedback along t so partial sums of the
    quantization error stay O(1 ulp) instead of growing as sqrt(T)."""
    out = np.empty(xb.shape, dtype=F8NP)
    acc = np.zeros(xb.shape[1], dtype=np.float32)
    for t in range(xb.shape[0]):
        q = (xb[t] - acc).astype(F8NP)
        out[t] = q
        acc += q.astype(np.float32) - xb[t]
    return out


def prepare(x, dispatch_mask, combine_array, w1, b1, w2):
    """Host-side prep: fp8 payloads in DR layouts + coherent-channel
    corrections. Returns (in_maps, corr) where corr[b] is the rank-1
    correction to add to batch b's output."""
    w1q = w1.astype(F8NP)
    w2q = w2.astype(F8NP)
    w1qf = w1q.astype(np.float32)
    w2qf = w2q.astype(np.float32)
    dw1 = w1 - w1qf   # [E, D, HE]
    dw2 = w2 - w2qf   # [E, HE, O]

    in_maps = []
    corr = [np.zeros((T, O), dtype=np.float32) for _ in range(B)]
    sub = np.arange(0, C, C // 256)
    for m in range(8):
        b, g = m // 2, m % 2
        es = [2 * g, 2 * g + 1]
        x8 = _sigma_delta_q8(x[b])                 # [T, D] fp8
        x8f = x8.astype(np.float32)
        xb_dev = np.ascontiguousarray(
            x8.reshape(nTP, 2, P, D).transpose(0, 2, 1, 3))

        dm_dev = np.empty((2, nTP, P, 2, C), dtype=F8NP)
        cmt_dev = np.empty((2, nCP, P, 2, T), dtype=F8NP)
        bb_host = np.empty((2, HE), dtype=np.float32)
        for ei, e in enumerate(es):
            dmq = dispatch_mask[b, :, e, :].astype(F8NP)     # [T, C]
            cmq = combine_array[b, :, e, :].astype(F8NP)     # [T, C]
            dm_dev[ei] = dmq.reshape(nTP, 2, P, C).transpose(0, 2, 1, 3)
            cmt_dev[ei] = np.ascontiguousarray(cmq.T).reshape(
                nCP, 2, P, T).transpose(0, 2, 1, 3)
            dmqf = dmq.astype(np.float32)
            cmqf = cmq.astype(np.float32)
            # biasB: m = mean_c(xd_dev) computed by commuting the c-sum
            mvec = (dmqf.sum(axis=1) / C) @ x8f              # [D]
            bb_host[ei] = b1[e] + mvec @ dw1[e]
            # w2 rank-1 correction: m_h from a 256-row subsample recompute
            xd_sub = (dmqf[:, sub].T @ x8f).astype(F8NP).astype(np.float32)
            a_sub = xd_sub @ w1qf[e] + bb_host[ei][None, :]
            from scipy.special import erf
            h_sub = a_sub * 0.5 * (1.0 + erf(a_sub / np.sqrt(2.0)))
            m_h = h_sub.mean(axis=0)                          # [HE]
            corr[b] += np.outer(cmqf.sum(axis=1), m_h @ dw2[e])

        # DR layouts for weights: d(or he) = kp*256 + i*128 + p
        w1_dev = np.ascontiguousarray(
            w1q[es].reshape(2, 2, 2, P, HE).transpose(3, 0, 1, 2, 4))
        w2_dev = np.ascontiguousarray(
            w2q[es].reshape(2, 2, 2, P, O).transpose(3, 0, 1, 2, 4))
        bb_dev = np.ascontiguousarray(
            bb_host.reshape(2, nMH, P).transpose(2, 0, 1))

        in_maps.append({
            "xb": xb_dev,
            "dm": np.ascontiguousarray(dm_dev),
            "cmt": np.ascontiguousarray(cmt_dev),
            "w1": w1_dev,
            "w2": w2_dev,
            "bb": bb_dev,
        })
    return in_maps, corr


def make_in_maps(x, dispatch_mask, combine_array, w1, b1, w2):
    return prepare(x, dispatch_mask, combine_array, w1, b1, w2)[0]


def kernel(x, dispatch_mask, combine_array, w1, b1, w2, b2):
    nc = get_nc()
    x, dispatch_mask, combine_array, w1, b1, w2 = (
        np.asarray(a, dtype=np.float32)
        for a in (x, dispatch_mask, combine_array, w1, b1, w2))
    in_maps, corr = prepare(x, dispatch_mask, combine_array, w1, b1, w2)
    res = bass_utils.run_bass_kernel_spmd(nc, in_maps, core_ids=list(range(8)))
    b2f = np.asarray(b2, dtype=np.float32)
    out = np.empty((B, T, O), dtype=np.float32)
    for b in range(B):
        out[b] = (res.results[2 * b]["pout"].astype(np.float32)
                  + res.results[2 * b + 1]["pout"].astype(np.float32)
                  + corr[b] + b2f)
    return out


# revision 29
# speedup vs baseline: 1.1612x; 1.0166x over previous
"""ExpertsChooseMlp Trainium2 kernel — all-fp8 DoubleRow pipeline.

Full inputs in, full output out. Sharding: 8 cores = 4 batches x 2 expert-pairs.
Core m handles batch b=m//2 and experts {2g, 2g+1}, g=m%2. Each core computes
pout[T,O] = sum_{e in pair} combine[b,:,e,:] @ mlp_e(dispatch[b,:,e,:]^T @ x[b]);
the host sums the two partials per batch, adds b2 and a rank-1 correction.

All four matmul phases run as fp8e4m3 DoubleRow (K=256/pass, 157 TF/s):
  A: xdT[D,C] = x8^T dm8        (K=T,  64 DR passes/expert)
  B: hT[HE,C] = gelu(w18^T xdT + biasB)   (K=D,  16 passes)
  C: y[C,O]   = hT^T w28        (K=HE, 16 passes)
  D: pout[T,O]= cmT^T y         (K=C,  64 passes over expert pair)
320 matmuls/core at 216ns = ~69us PE-busy.

fp8 accuracy (meas. ~5.7e-3 max rel vs fp32 reference, gate 2e-2) relies on
three host-side corrections that cost zero device time:
 1. x is quantized with sigma-delta error feedback along t: the coherent
    channel (all-positive dispatch weights ~0.5 make the output dominated by
    t-sums of x) would otherwise amplify the sqrt(T) random walk of plain
    rounding error into ~2.4e-2.
 2. w1's quantization error rides the same coherent channel (it multiplies the
    c-mean of xd, constant across capacity slots). biasB = b1 + m @ (w1 - q8(w1))
    with m = mean_c(xd) = (rowsum(dm8)/C) @ x8, computed exactly on host.
 3. w2: same mechanism via the c-mean of h; gelu blocks commuting the mean, so
    m_h is estimated from a 256-row subsample of a host recompute, and the
    correction enters as a host-side rank-1 update R_cm (x) m_h@(w2-q8(w2)).
"""
import sys

sys.path.insert(0, "/opt/trn_rl_repo")

import numpy as np
import ml_dtypes

import concourse.bacc as bacc
import concourse.mybir as mybir
import concourse.tile as tile
from concourse import bass_utils

B, T, D, E, C, HE, O = 4, 2048, 512, 4, 1024, 512, 512
P = 128
nTP = T // 256    # 8  DR passes over T
nMD = D // P      # 4  D-chunks
nMH = HE // P     # 4  HE-chunks
nCC = C // P      # 8  C-chunks
nMT = T // P      # 16 T-chunks (phase D output)
NF = 512          # matmul free dim (one PSUM bank)

F32 = mybir.dt.float32
BF16 = mybir.dt.bfloat16
F8 = mybir.dt.float8e4
GELU = mybir.ActivationFunctionType.Gelu
COPY = mybir.ActivationFunctionType.Copy
DR = mybir.MatmulPerfMode.DoubleRow
nCP = nCC // 2    # 4  C pair-chunks for phase D (K=256 per matmul)

F8NP = ml_dtypes.float8_e4m3

_NC = None


def _build():
    nc = bacc.Bacc("TRN2", target_bir_lowering=False, debug=False,
                   enable_asserts=False, num_devices=1)
    # x8 in DR layout: t = tp*256 + i*128 + p  ->  xb[tp, p, i, d]
    xb = nc.dram_tensor("xb", [nTP, P, 2, D], F8, kind="ExternalInput").ap()
    # dm in DR layout per expert: dm[e, tp, p, i, c]
    dm = nc.dram_tensor("dm", [2, nTP, P, 2, C], F8, kind="ExternalInput").ap()
    # cmt planes for phase D: c = kp*256 + i*128 + p -> cmt[e, kp, p, i, t]
    cmt = nc.dram_tensor("cmt", [2, nCP, P, 2, T], F8, kind="ExternalInput").ap()
    # w1 in DR layout: d = pass*256 + i*128 + p -> w1[p, e, pass, i, he]
    w1 = nc.dram_tensor("w1", [P, 2, 2, 2, HE], F8, kind="ExternalInput").ap()
    # w2 in DR layout: he = pass*256 + i*128 + p -> w2[p, e, pass, i, o]
    w2 = nc.dram_tensor("w2", [P, 2, 2, 2, O], F8, kind="ExternalInput").ap()
    # biasB[p, e, mh] = b1 + m@(w1-q8(w1)), he = mh*128 + p
    bb = nc.dram_tensor("bb", [P, 2, nMH], F32, kind="ExternalInput").ap()
    pout = nc.dram_tensor("pout", [T, O], BF16, kind="ExternalOutput").ap()

    with tile.TileContext(nc) as tc:
        with (
            tc.tile_pool(name="const", bufs=1) as const,
            tc.tile_pool(name="dmp", bufs=16) as dmp,
            tc.tile_pool(name="cmp", bufs=8) as cmp_,
            tc.tile_pool(name="inter", bufs=1) as inter,
            tc.tile_pool(name="yp", bufs=2) as yp,
            tc.tile_pool(name="outp", bufs=4) as outp,
            tc.tile_pool(name="psum", bufs=8, space="PSUM") as psp,
        ):
            # ---- warmup memset on Vector so the DMA-capable rings aren't
            # blocked behind it ----
            warm = const.tile([P, NF], BF16)
            nc.vector.memset(warm[:], 0.0)
            # preload the gelu activation table now (it otherwise loads lazily
            # at the first phase-B ACTIVATE, stalling it ~1.3us)
            warm8 = const.tile([P, 1], F8)
            nc.scalar.activation(warm8[:], warm[:, 0:1], GELU)

            # ---- DMA ring assignment. Queues pull greedily and contend for
            # the ~358GB/s HBM port, so later-needed bulk must sit BEHIND
            # earlier-needed bulk on the same queue (FIFO paces it), not on a
            # parallel queue where it would starve the critical stream:
            #   sync   : dm[e0] -> dm[e1] -> cmt[e0] -> cmt[e1]  (need-order)
            #   scalar : x8 -> weights/bias -> (idle) -> pout writes
            x_sb = const.tile([P, nTP, 2, D], F8)
            nc.scalar.dma_start(x_sb[:, 0, :, 0:D // 2], xb[0][:, :, 0:D // 2])
            nc.scalar.dma_start(x_sb[:, 0, :, D // 2:D], xb[0][:, :, D // 2:D])
            for tp in range(1, nTP):
                nc.scalar.dma_start(x_sb[:, tp, :, :], xb[tp])
            dm_t0 = []
            for tp in range(nTP):
                t_ = dmp.tile([P, 2, C], F8, tag="dm")
                if tp < 2:
                    # split the first tiles so the first matmuls start as soon
                    # as a half-tile (128KB) lands instead of a full 256KB
                    nc.sync.dma_start(t_[:, :, 0:NF], dm[0, tp][:, :, 0:NF])
                    nc.sync.dma_start(t_[:, :, NF:C], dm[0, tp][:, :, NF:C])
                else:
                    nc.sync.dma_start(t_[:], dm[0, tp])
                dm_t0.append(t_)
            w1_sb = const.tile([P, 2, 2, 2, HE], F8)
            nc.scalar.dma_start(w1_sb[:], w1)
            w2_sb = const.tile([P, 2, 2, 2, O], F8)
            nc.scalar.dma_start(w2_sb[:], w2)
            bb_sb = const.tile([P, 2 * nMH], F32)
            nc.scalar.dma_start(bb_sb[:], bb.rearrange("p e mh -> p (e mh)"))
            dm_t1 = []
            for tp in range(nTP):
                t_ = dmp.tile([P, 2, C], F8, tag="dm")
                nc.sync.dma_start(t_[:], dm[1, tp])
                dm_t1.append(t_)
            cmt_t = {}
            for ei in range(2):
                for kp in range(nCP):
                    t_ = cmp_.tile([P, 2, T], F8, tag="cmt")
                    nc.sync.dma_start(t_[:], cmt[ei, kp])
                    cmt_t[(ei, kp)] = t_

            # ---- HAM warmup: dummy matmuls on zeroed SBUF during the initial
            # DMA wait so real matmuls start at 2.4GHz. ----
            ps_w = psp.tile([P, NF], F32, tag="ps", name="ps_warm")
            NWARM = 6
            for i in range(NWARM):
                nc.tensor.matmul(ps_w[:], warm[:, 0:P], warm[:],
                                 start=(i == 0), stop=(i == NWARM - 1))

            y_tiles = []
            for ei in range(2):
                dm_t = dm_t0 if ei == 0 else dm_t1

                # ---- phase A: xdT[D, C] = x8^T dm8 (fp8 DR) ----
                # tp-outer: all 8 PSUM banks accumulate in parallel, each dm
                # tile consumed once and released for the next expert prefetch.
                # The final pass runs bank-by-bank (ncc0 banks first) with the
                # drain issued right behind each stop, split across Vector and
                # GpSimd, so phase B starts ~0.5us after A's last matmul
                # instead of waiting for 8 serialized casts.
                xdt = inter.tile([P, nMD, C], F8, tag="xdt")
                pss = [psp.tile([P, NF], F32, tag="ps", name=f"psa{i}")
                       for i in range(2 * nMD)]
                for tp in range(nTP - 1):
                    for ncc in range(2):
                        for mc in range(nMD):
                            nc.tensor.matmul(
                                pss[2 * mc + ncc][:],
                                x_sb[:, tp, :, mc * P:(mc + 1) * P],
                                dm_t[tp][:, :, ncc * NF:(ncc + 1) * NF],
                                start=(tp == 0), stop=False, perf_mode=DR)
                for ncc in range(2):
                    for mc in range(nMD):
                        nc.tensor.matmul(
                            pss[2 * mc + ncc][:],
                            x_sb[:, nTP - 1, :, mc * P:(mc + 1) * P],
                            dm_t[nTP - 1][:, :, ncc * NF:(ncc + 1) * NF],
                            start=False, stop=True, perf_mode=DR)
                        # alternate drains Vector/Scalar per chunk so phase
                        # B's pass0 (chunks 0+1) is ready ~0.7us after A's
                        # last matmul (GpSimd cannot read PSUM)
                        sl_ = slice(ncc * NF, (ncc + 1) * NF)
                        if mc % 2 == 0:
                            nc.vector.tensor_copy(xdt[:, mc, sl_],
                                                  pss[2 * mc + ncc][:])
                        else:
                            nc.scalar.activation(xdt[:, mc, sl_],
                                                 pss[2 * mc + ncc][:], COPY)

                # ---- phase B: hT[HE, C] = gelu(w18^T xdT + biasB) (fp8 DR) ----
                # ncc-outer; within an ncc, kp-outer across 4 banks so the
                # first 4 matmuls only need xdt chunks 0-1.
                ht = inter.tile([P, nMH, C], F8, tag="ht")
                for ncc in range(2):
                    sl = slice(ncc * NF, (ncc + 1) * NF)
                    psB = [psp.tile([P, NF], F32, tag="ps",
                                    name=f"psb{ei}{ncc}{i}") for i in range(nMH)]
                    for kp in range(2):
                        for mh in range(nMH):
                            nc.tensor.matmul(
                                psB[mh][:],
                                w1_sb[:, ei, kp, :, mh * P:(mh + 1) * P],
                                xdt[:, 2 * kp:2 * kp + 2, sl],
                                start=(kp == 0), stop=(kp == 1), perf_mode=DR)
                    for mh in range(nMH):
                        bia = bb_sb[:, ei * nMH + mh:ei * nMH + mh + 1]
                        nc.scalar.activation(ht[:, mh, sl], psB[mh][:], GELU,
                                             bias=bia)

                # ---- phase C: y[C, O] = hT^T w28 (fp8 DR), stored in DR plane
                # layout for phase D: row c = kp*256 + i*128 + p -> y_sb[p,kp,i,:]
                y_sb = yp.tile([P, nCP, 2, O], F8, tag="y")
                for cc in range(nCC):
                    ps = psp.tile([P, NF], F32, tag="ps")
                    for kp in range(2):
                        nc.tensor.matmul(ps[:],
                                         ht[:, 2 * kp:2 * kp + 2, cc * P:(cc + 1) * P],
                                         w2_sb[:, ei, kp, :, :],
                                         start=(kp == 0), stop=(kp == 1),
                                         perf_mode=DR)
                    if cc % 2 == 0:
                        nc.vector.tensor_copy(y_sb[:, cc // 2, 0, :], ps[:])
                    else:
                        nc.scalar.activation(y_sb[:, cc // 2, 1, :], ps[:], COPY)
                y_tiles.append(y_sb)

            # ---- phase D: pout[T, O] = sum_e cmT_e^T y_e (fp8 DR) ----
            for mt in range(nMT):
                ps = psp.tile([P, NF], F32, tag="ps")
                idx = 0
                for ei in range(2):
                    for kp in range(nCP):
                        nc.tensor.matmul(ps[:],
                                         cmt_t[(ei, kp)][:, :, mt * P:(mt + 1) * P],
                                         y_tiles[ei][:, kp, :, :],
                                         start=(idx == 0), stop=(idx == 7),
                                         perf_mode=DR)
                        idx += 1
                ot = outp.tile([P, O], BF16, tag="out")
                rows = slice(mt * P, (mt + 1) * P)
                if mt < nMT - 1:
                    nc.vector.tensor_copy(ot[:], ps[:])
                    nc.scalar.dma_start(pout[rows, :], ot[:])
                else:
                    # split the last drain across engines/queues to shorten
                    # the tail after the final matmul
                    nc.vector.tensor_copy(ot[:, 0:O // 2], ps[:, 0:O // 2])
                    nc.scalar.activation(ot[:, O // 2:O], ps[:, O // 2:O], COPY)
                    nc.sync.dma_start(pout[rows, 0:O // 2], ot[:, 0:O // 2])
                    nc.scalar.dma_start(pout[rows, O // 2:O], ot[:, O // 2:O])

    nc.compile()
    return nc


def get_nc():
    global _NC
    if _NC is None:
        _NC = _build()
    return _NC


def _sigma_delta_q8(xb):
    """fp8 quantization with error feedback along t so partial sums of the
    quantization error stay O(1 ulp) instead of growing as sqrt(T)."""
    out = np.empty(xb.shape, dtype=F8NP)
    acc = np.zeros(xb.shape[1], dtype=np.float32)
    for t in range(xb.shape[0]):
        q = (xb[t] - acc).astype(F8NP)
        out[t] = q
        acc += q.astype(np.float32) - xb[t]
    return out


def prepare(x, dispatch_mask, combine_array, w1, b1, w2):
    """Host-side prep: fp8 payloads in DR layouts + coherent-channel
    corrections. Returns (in_maps, corr) where corr[b] is the rank-1
    correction to add to batch b's output."""
    w1q = w1.astype(F8NP)
    w2q = w2.astype(F8NP)
    w1qf = w1q.astype(np.float32)
    w2qf = w2q.astype(np.float32)
    dw1 = w1 - w1qf   # [E, D, HE]
    dw2 = w2 - w2qf   # [E, HE, O]

    in_maps = []
    corr = [np.zeros((T, O), dtype=np.float32) for _ in range(B)]
    sub = np.arange(0, C, C // 256)
    for m in range(8):
        b, g = m // 2, m % 2
        es = [2 * g, 2 * g + 1]
        x8 = _sigma_delta_q8(x[b])                 # [T, D] fp8
        x8f = x8.astype(np.float32)
        xb_dev = np.ascontiguousarray(
            x8.reshape(nTP, 2, P, D).transpose(0, 2, 1, 3))

        dm_dev = np.empty((2, nTP, P, 2, C), dtype=F8NP)
        cmt_dev = np.empty((2, nCP, P, 2, T), dtype=F8NP)
        bb_host = np.empty((2, HE), dtype=np.float32)
        for ei, e in enumerate(es):
            dmq = dispatch_mask[b, :, e, :].astype(F8NP)     # [T, C]
            cmq = combine_array[b, :, e, :].astype(F8NP)     # [T, C]
            dm_dev[ei] = dmq.reshape(nTP, 2, P, C).transpose(0, 2, 1, 3)
            cmt_dev[ei] = np.ascontiguousarray(cmq.T).reshape(
                nCP, 2, P, T).transpose(0, 2, 1, 3)
            dmqf = dmq.astype(np.float32)
            cmqf = cmq.astype(np.float32)
            # biasB: m = mean_c(xd_dev) computed by commuting the c-sum
            mvec = (dmqf.sum(axis=1) / C) @ x8f              # [D]
            bb_host[ei] = b1[e] + mvec @ dw1[e]
            # w2 rank-1 correction: m_h from a 256-row subsample recompute
            xd_sub = (dmqf[:, sub].T @ x8f).astype(F8NP).astype(np.float32)
            a_sub = xd_sub @ w1qf[e] + bb_host[ei][None, :]
            from scipy.special import erf
            h_sub = a_sub * 0.5 * (1.0 + erf(a_sub / np.sqrt(2.0)))
            m_h = h_sub.mean(axis=0)                          # [HE]
            corr[b] += np.outer(cmqf.sum(axis=1), m_h @ dw2[e])

        # DR layouts for weights: d(or he) = kp*256 + i*128 + p
        w1_dev = np.ascontiguousarray(
            w1q[es].reshape(2, 2, 2, P, HE).transpose(3, 0, 1, 2, 4))
        w2_dev = np.ascontiguousarray(
            w2q[es].reshape(2, 2, 2, P, O).transpose(3, 0, 1, 2, 4))
        bb_dev = np.ascontiguousarray(
            bb_host.reshape(2, nMH, P).transpose(2, 0, 1))

        in_maps.append({
            "xb": xb_dev,
            "dm": np.ascontiguousarray(dm_dev),
            "cmt": np.ascontiguousarray(cmt_dev),
            "w1": w1_dev,
            "w2": w2_dev,
            "bb": bb_dev,
        })
    return in_maps, corr


def make_in_maps(x, dispatch_mask, combine_array, w1, b1, w2):
    return prepare(x, dispatch_mask, combine_array, w1, b1, w2)[0]


def kernel(x, dispatch_mask, combine_array, w1, b1, w2, b2):
    nc = get_nc()
    x, dispatch_mask, combine_array, w1, b1, w2 = (
        np.asarray(a, dtype=np.float32)
        for a in (x, dispatch_mask, combine_array, w1, b1, w2))
    in_maps, corr = prepare(x, dispatch_mask, combine_array, w1, b1, w2)
    res = bass_utils.run_bass_kernel_spmd(nc, in_maps, core_ids=list(range(8)))
    b2f = np.asarray(b2, dtype=np.float32)
    out = np.empty((B, T, O), dtype=np.float32)
    for b in range(B):
        out[b] = (res.results[2 * b]["pout"].astype(np.float32)
                  + res.results[2 * b + 1]["pout"].astype(np.float32)
                  + corr[b] + b2f)
    return out
